# revision 4
# baseline (speedup 1.0000x reference)
"""Trainium2 Bass kernel for nn_Attn_59210419142961 (MLP-attention).

Reference computation (per batch b):
    q = inp @ u_w.T                       # [L, D]
    k = inp @ v_w.T                       # [L, D]
    score[i, j] = sum_d a_d * tanh(q[j, d] + k[i, d])     # [L, L]
    attn = softmax(score, axis=-1)
    out = attn @ inp                      # [L, D]
returns (out, attn).

Sharding: pure data-parallel over batch B=8 across the 8 NeuronCores
(one batch per core, weights replicated). No collectives.

Per-core design (L=512, D=128):
  - preamble: PE-transpose inp -> inp_T [d, l]; u_w,v_w -> u_T,v_T [d, e];
    q_T = u_T.T @ inp_T, k_T = v_T.T @ inp_T  (both [e=128, l=512] in SBUF)
  - phase 1 (the 33.5M-element tanh, ACT-bound):
      for each query row i: z[:, i-slot] = q_T + k_T[:, i]  (DVE tensor_scalar,
      per-partition scalar broadcast), batched CH rows per [128, CH*512] tile;
      one big ACT Tanh per tile (f32 in -> bf16 out) amortizes the ~352-cycle
      ACTIVATE overhead; per-row PE matmul with stationary a [128,1] reduces
      over d into one PSUM partition row -> score block [128 i, 512 j].
  - phase 2: softmax over j on the free axis (no max-subtraction needed:
    |score| <= sum|a_d| ~ 11, exp is safe in f32), attn -> DRAM, and
    out = attn @ inp via PE-transposed attn blocks accumulated in PSUM.
"""

import numpy as np

L = 512
D = 128
N_CORES = 8
CH = 16  # query rows per z-tile; ACT tile free dim = CH*512

_CACHE = {}


def _build_nc():
    import concourse.bacc as bacc
    import concourse.mybir as mybir
    import concourse.tile as tile
    import concourse.masks as masks

    f32 = mybir.dt.float32
    bf16 = mybir.dt.bfloat16
    AF = mybir.ActivationFunctionType

    nc = bacc.Bacc("TRN2", target_bir_lowering=False, debug=False,
                   num_devices=N_CORES)

    inp_d = nc.declare_dram_parameter("inp", [L, D], f32, isOutput=False)
    u_d = nc.declare_dram_parameter("u_w", [D, D], f32, isOutput=False)
    v_d = nc.declare_dram_parameter("v_w", [D, D], f32, isOutput=False)
    a_d = nc.declare_dram_parameter("a_w", [1, D], f32, isOutput=False)
    out_d = nc.declare_dram_parameter("out", [L, D], f32, isOutput=True)
    attn_d = nc.declare_dram_parameter("attn", [L, L], f32, isOutput=True)

    NBLK = L // 128          # 4 i-blocks of 128 rows
    PER_BLK = 128 // CH      # chunks per i-block

    with tile.TileContext(nc) as tc:
        with (
            tc.tile_pool(name="const", bufs=1) as cpool,
            tc.tile_pool(name="zpool", bufs=2) as zpool,
            tc.tile_pool(name="tpool", bufs=2) as tpool,
            tc.tile_pool(name="score", bufs=1) as spool,
            tc.tile_pool(name="work", bufs=3) as wpool,
            tc.tile_pool(name="psum", bufs=3, space="PSUM") as ppool,
            tc.tile_pool(name="psum_s", bufs=2, space="PSUM") as pspool,
        ):
            ident = cpool.tile([128, 128], f32)
            masks.make_identity(nc, ident[:])

            # ---- load inputs ----
            inp_sb = cpool.tile([128, NBLK, 128], f32)  # [p, c, d]; l = c*128+p
            for c in range(NBLK):
                nc.sync.dma_start(inp_sb[:, c, :], inp_d[c * 128:(c + 1) * 128, :])
            u_sb = cpool.tile([128, 128], f32)
            nc.sync.dma_start(u_sb[:], u_d[:, :])
            v_sb = cpool.tile([128, 128], f32)
            nc.sync.dma_start(v_sb[:], v_d[:, :])
            a_f32 = cpool.tile([128, 1], f32)
            nc.sync.dma_start(a_f32[:], a_d[:, :].rearrange("a d -> d a"))
            # a embedded in zeros at column 127: the window
            # a_embed[:, 127-m : 255-m] is a [128,128] matrix whose only
            # nonzero column is m. Used as stationary lhsT so query m's
            # d-reduction lands on PSUM partition row m (PE can only base
            # matmul outputs at partition 0/32/64).
            a_embed = cpool.tile([128, 256], bf16)
            nc.gpsimd.memset(a_embed[:], 0.0)
            nc.vector.tensor_copy(a_embed[:, 127:128], a_f32[:])

            # ---- transposes: inp_T [d, l], u_T/v_T [d, e] ----
            inp_T = cpool.tile([128, L], f32)
            for c in range(NBLK):
                pt = ppool.tile([128, 128], f32, tag="pp")
                nc.tensor.transpose(pt[:], inp_sb[:, c, :], ident[:])
                nc.vector.tensor_copy(inp_T[:, c * 128:(c + 1) * 128], pt[:])
            u_T = cpool.tile([128, 128], f32)
            pt = ppool.tile([128, 128], f32, tag="pp")
            nc.tensor.transpose(pt[:], u_sb[:], ident[:])
            nc.vector.tensor_copy(u_T[:], pt[:])
            v_T = cpool.tile([128, 128], f32)
            pt = ppool.tile([128, 128], f32, tag="pp")
            nc.tensor.transpose(pt[:], v_sb[:], ident[:])
            nc.vector.tensor_copy(v_T[:], pt[:])

            # ---- q_T, k_T = (u_w @ inp_T), (v_w @ inp_T)  [e=128, l=512] ----
            q_T = cpool.tile([128, L], f32)
            pq = ppool.tile([128, L], f32, tag="pp")
            nc.tensor.matmul(pq[:], u_T[:], inp_T[:])
            nc.vector.tensor_copy(q_T[:], pq[:])
            k_T = cpool.tile([128, L], f32)
            pk = ppool.tile([128, L], f32, tag="pp")
            nc.tensor.matmul(pk[:], v_T[:], inp_T[:])
            nc.vector.tensor_copy(k_T[:], pk[:])

            # ---- phase 1: score[i, j] blocks ----
            score_sb = spool.tile([128, NBLK, L], f32)  # [i%128, i//128, j]
            for blk in range(NBLK):
                ps = pspool.tile([128, L], f32)
                for cc in range(PER_BLK):
                    z = zpool.tile([128, CH * L], f32)
                    t = tpool.tile([128, CH * L], bf16)
                    for il in range(CH):
                        i = blk * 128 + cc * CH + il
                        nc.vector.tensor_scalar_add(
                            z[:, il * L:(il + 1) * L], q_T[:], k_T[:, i:i + 1])
                    nc.scalar.activation(t[:], z[:], AF.Tanh)
                    for il in range(CH):
                        i_loc = cc * CH + il
                        nc.tensor.matmul(
                            ps[:], a_embed[:, 127 - i_loc:255 - i_loc],
                            t[:, il * L:(il + 1) * L],
                            start=(i_loc == 0), stop=(i_loc == 127))
                nc.vector.tensor_copy(score_sb[:, blk, :], ps[:])

            # ---- phase 2: softmax + attn @ inp ----
            for blk in range(NBLK):
                e_sb = wpool.tile([128, L], f32)
                nc.scalar.activation(e_sb[:], score_sb[:, blk, :], AF.Exp)
                ssum = wpool.tile([128, 1], f32)
                nc.vector.reduce_sum(ssum[:], e_sb[:], axis=mybir.AxisListType.X)
                rec = wpool.tile([128, 1], f32)
                nc.vector.reciprocal(rec[:], ssum[:])
                attn_sb = wpool.tile([128, L], f32)
                nc.vector.tensor_scalar_mul(attn_sb[:], e_sb[:], rec[:])
                nc.sync.dma_start(attn_d[blk * 128:(blk + 1) * 128, :], attn_sb[:])

                aTs = []
                for jc in range(NBLK):
                    ptr = ppool.tile([128, 128], f32, tag="pp")
                    nc.tensor.transpose(
                        ptr[:], attn_sb[:, jc * 128:(jc + 1) * 128], ident[:])
                    aT = wpool.tile([128, 128], f32, tag="attn_T")
                    nc.vector.tensor_copy(aT[:], ptr[:])
                    aTs.append(aT)
                po = ppool.tile([128, 128], f32, tag="pp")
                for jc in range(NBLK):
                    nc.tensor.matmul(po[:], aTs[jc][:], inp_sb[:, jc, :],
                                     start=(jc == 0), stop=(jc == NBLK - 1))
                ob = wpool.tile([128, 128], f32)
                nc.vector.tensor_copy(ob[:], po[:])
                nc.sync.dma_start(out_d[blk * 128:(blk + 1) * 128, :], ob[:])

    nc.compile()
    return nc


def get_nc():
    if "nc" not in _CACHE:
        _CACHE["nc"] = _build_nc()
    return _CACHE["nc"]


def run_hw(in_maps, trace=False):
    from concourse.bass_utils import run_bass_kernel_spmd
    return run_bass_kernel_spmd(get_nc(), in_maps, core_ids=list(range(N_CORES)),
                                trace=trace)


def make_in_maps(inp, u_w, v_w, a_w):
    inp = np.ascontiguousarray(inp, dtype=np.float32)
    u_w = np.ascontiguousarray(u_w, dtype=np.float32)
    v_w = np.ascontiguousarray(v_w, dtype=np.float32)
    a_w = np.ascontiguousarray(a_w, dtype=np.float32)
    return [{"inp": inp[b], "u_w": u_w, "v_w": v_w, "a_w": a_w}
            for b in range(N_CORES)]


def kernel(inp, u_w, v_w, a_w):
    res = run_hw(make_in_maps(inp, u_w, v_w, a_w), trace=False)
    out = np.stack([res.results[b]["out"] for b in range(N_CORES)])
    attn = np.stack([res.results[b]["attn"] for b in range(N_CORES)])
    return out, attn
